# revision 21
# baseline (speedup 1.0000x reference)
"""Trainium2 Bass kernel for nn_DeRNN_4054449127979.

Network (per reference):
  stage1: 6 shared-weight single-channel LSTMs (hidden 16) over T=1024,
          folded as one LSTM on [B*6, T, 1]; keep last hidden -> feat [B, 96]
  stage2: LSTM(1 -> 128) over the 96 features as a sequence (return_seq)
  stage3: LSTM(128 -> 128) over those 96 steps; keep last hidden
  head:   relu(fc1) -> relu(fc3) -> fc2  -> [B, 2]

Sharding: pure data parallel over batch across 8 cores (B=2048 -> 256/core).

Stage-1 design:
  - all matmuls bf16 (HW runs fp32/f32r matmuls at 4 cyc/row; bf16 at 1)
  - x rows + ones row live inside the state tile -> ONE combined matmul
    per gate (K=103) instead of separate x/h matmuls; the per-step x copy
    runs in the tanh(c)/h window, off the critical chain
  - sigmoid-via-tanh: all gates + tanh(c) use ONE activation function
    (Tanh, scale=0.5): i,f,o weights plain (tanh(z/2) -> sigma=(t+1)/2),
    g-gate weights doubled, cell state stored as c2=2c, hidden stored as
    h2=2h with Whh halved.  One act table, 2 calls/step/stream.
  - cell math as fused scalar_tensor_tensor / tensor_scalar ops
  - batch split in two j-streams (cols 0:128 / 128:256) whose per-step
    chains interleave on the engines.
Stage 2/3: baseline wavefront structure, bf16 matmuls, bias folded into
the K=2 x-matmul (stage2).
"""

import sys

import numpy as np

sys.path.insert(0, "/opt/trn_rl_repo")

import concourse.bass as bass  # noqa: E402
import concourse.tile as tile  # noqa: E402
from concourse import bacc, mybir  # noqa: E402

F32 = mybir.dt.float32
BF16 = mybir.dt.bfloat16
AF = mybir.ActivationFunctionType
ALU = mybir.AluOpType

B = 2048
NCORES = 8
BC = B // NCORES  # 256
HB = BC // 2  # 128 per j-stream
NCH = 6
H1 = 16
G1 = NCH * H1  # 96
H2 = 128
T1_FULL = 1024
T2_FULL = 96
CH = 32  # stage-1 steps per staged x chunk
SROWS = 103  # state tile rows: 0:96 h2 | 96:102 x | 102 ones

# stage-1 gate bank order on chip: f, i, g, o ; torch rows are i, f, g, o
GATE_BASES_1 = (H1, 0, 2 * H1, 3 * H1)
# stage-2/3 bank order: i, f, o, g
GATE_BASES_2 = (0, H2, 3 * H2, 2 * H2)

# ---- packed bf16 weight column map ----
_off = 0


def _take(n):
    global _off
    o = _off
    _off += n
    return o


W1_O = _take(4 * G1)  # 4 gates x [SROWS, 96]
W2X_O = _take(4 * H2)  # rows 0:2 = [Wih0/2 ; b2]
W2H_O = _take(4 * H2)  # [128, 128] per gate
W3X_O = _take(4 * H2)  # [128, 128] per gate (Wih1^T)
W3H_O = _take(4 * H2)  # [128, 128] per gate
B3_O = _take(4 * H2)  # row 0 = b3 per gate
WF1_O = _take(H2)
WF3_O = _take(H2)
WF2_O = _take(2)
ONES_O = _take(BC)  # row 0 = ones
WBCOLS = _off


def _r(ap, pattern, **kw):
    return ap.rearrange(pattern, **kw)


def build_program(T1=T1_FULL, T2=T2_FULL, ch=CH):
    nc = bacc.Bacc("TRN2", target_bir_lowering=False)

    tpad = 2 * ch
    x_t = nc.declare_dram_parameter("xT", [NCH, T1 + tpad, BC], BF16, isOutput=False)
    wp_d = nc.declare_dram_parameter("wpack", [128, WBCOLS], BF16, isOutput=False)
    wf_d = nc.declare_dram_parameter("wpf", [128, 3], F32, isOutput=False)
    y_t = nc.declare_dram_parameter("yT", [2, BC], F32, isOutput=True)
    fdbg = nc.declare_dram_parameter("fdbg", [G1, BC], BF16, isOutput=True)

    feat_d = nc.dram_tensor("featstage", [G1 + 32, BC], BF16)

    def mm(out, lhsT, rhs, start, stop):
        nc.tensor.matmul(out, lhsT, rhs, start=start, stop=stop)

    with tile.TileContext(nc) as tc:
        with (
            tc.tile_pool(name="wpool", bufs=1) as wpool,
            tc.tile_pool(name="state", bufs=1) as state,
            tc.tile_pool(name="work", bufs=3) as work,
        ):
            wp = wpool.tile([128, WBCOLS], BF16)
            nc.sync.dma_start(wp[:], wp_d[:])
            wpf = wpool.tile([128, 3], F32)
            nc.sync.dma_start(wpf[:], wf_d[:])

            w1 = [wp[0:SROWS, W1_O + G1 * t : W1_O + G1 * (t + 1)] for t in range(4)]
            w2x = [wp[0:2, W2X_O + H2 * t : W2X_O + H2 * (t + 1)] for t in range(4)]
            w2h = [wp[0:H2, W2H_O + H2 * t : W2H_O + H2 * (t + 1)] for t in range(4)]
            w3x = [wp[0:H2, W3X_O + H2 * t : W3X_O + H2 * (t + 1)] for t in range(4)]
            w3h = [wp[0:H2, W3H_O + H2 * t : W3H_O + H2 * (t + 1)] for t in range(4)]
            b3 = [wp[0:1, B3_O + H2 * t : B3_O + H2 * (t + 1)] for t in range(4)]
            wf1 = wp[0:H2, WF1_O : WF1_O + H2]
            wf3 = wp[0:H2, WF3_O : WF3_O + H2]
            wf2 = wp[0:H2, WF2_O : WF2_O + 2]
            ones1 = wp[0:1, ONES_O : ONES_O + BC]
            bf1 = wpf[0:H2, 0:1]
            bf3 = wpf[0:H2, 1:2]
            bf2 = wpf[0:2, 2:3]

            # ---------------- stage 1 ----------------
            S = state.tile([SROWS, BC], BF16)  # h2 | x | ones
            nc.vector.memset(S[:], 0.0)
            # ones row: memset rows 96:103 (quadrant-aligned start); the
            # x rows 96:102 are overwritten by the first xcopy anyway
            nc.vector.memset(S[96:SROWS, :], 1.0)
            c2 = state.tile([G1, BC], F32)
            nc.vector.memset(c2[:], 0.0)
            th = [state.tile([G1, 512], BF16, name=f"th_{s}") for s in range(2)]
            thc = [state.tile([G1, HB], BF16, name=f"thc_{s}") for s in range(2)]
            for s in range(2):
                nc.vector.memset(th[s][:], 0.0)
                nc.vector.memset(thc[s][:], 0.0)
            tf2 = [work.tile([G1, HB], BF16, name=f"tf2_{s}", tag="tf2") for s in range(2)]
            p1t = [work.tile([G1, HB], BF16, name=f"p1_{s}", tag="p1") for s in range(2)]
            p2t = [work.tile([G1, HB], F32, name=f"p2_{s}", tag="p2") for s in range(2)]

            with tc.tile_pool(name="ps1pool", bufs=1, space="PSUM") as ps1pool:
                ps1 = [ps1pool.tile([128, 512], F32, name=f"ps1_{s}") for s in range(2)]
                xv = _r(x_t[:], "c t b -> c (t b)")

                def xdma(tidx):
                    # DMA x rows for step tidx straight into the state tile;
                    # completes in the thc/h2 window, off the matmul chain.
                    nc.sync.dma_start(
                        S[96:102, :], xv[:, bass.ds(tidx * BC, BC)]
                    )

                def substep(s):
                    # bank order: 0=f, 1=i, 2=g, 3=o
                    js = slice(s * HB, (s + 1) * HB)
                    # tanh(c(t-1)) for this stream (c2 = 2c, scale 0.5)
                    nc.scalar.activation(thc[s][:], c2[:, js], AF.Tanh, scale=0.5)
                    # h2(t-1) = (t_o + 1) * thc -> state rows (bf16)
                    nc.vector.scalar_tensor_tensor(
                        S[0:G1, js], th[s][:, 384:512], 1.0, thc[s][:],
                        op0=ALU.add, op1=ALU.mult,
                    )
                    # gates(t): 4 combined matmuls K=SROWS, N=HB; acts split
                    # [f,i] / [g,o] so the first tanh overlaps the g/o mms
                    for t4 in range(2):
                        mm(
                            ps1[s][0:G1, t4 * 128 : t4 * 128 + 128],
                            w1[t4],
                            S[0:SROWS, js],
                            True,
                            True,
                        )
                    nc.scalar.activation(
                        _r(th[s][:, 0:256], "p (t b) -> p t b", b=128),
                        _r(ps1[s][0:G1, 0:256], "p (t b) -> p t b", b=128),
                        AF.Tanh,
                        scale=0.5,
                    )
                    for t4 in range(2, 4):
                        mm(
                            ps1[s][0:G1, t4 * 128 : t4 * 128 + 128],
                            w1[t4],
                            S[0:SROWS, js],
                            True,
                            True,
                        )
                    nc.scalar.activation(
                        _r(th[s][:, 256:512], "p (t b) -> p t b", b=128),
                        _r(ps1[s][0:G1, 256:512], "p (t b) -> p t b", b=128),
                        AF.Tanh,
                        scale=0.5,
                    )
                    # cell: sigma_f = tf*0.5+0.5 (Pool) ; P1 = (ti+1)*tg ;
                    # P2 = sigma_f * c2 ; c2' = P1 + P2
                    nc.gpsimd.tensor_scalar(
                        tf2[s][:], th[s][:, 0:128], 0.5, 0.5,
                        op0=ALU.mult, op1=ALU.add,
                    )
                    nc.vector.scalar_tensor_tensor(
                        p1t[s][:], th[s][:, 128:256], 1.0, th[s][:, 256:384],
                        op0=ALU.add, op1=ALU.mult,
                    )
                    nc.vector.tensor_tensor(
                        p2t[s][:], tf2[s][:], c2[:, js], op=ALU.mult
                    )
                    nc.vector.tensor_tensor(
                        c2[:, js], p1t[s][:], p2t[s][:], op=ALU.add
                    )

                nchunks = T1 // ch
                xdma(0)  # x rows for step 0

                def chunk_pair(ivc):
                    for k in range(2):
                        for jj in range(ch):
                            substep(0)
                            substep(1)
                            # x rows for the next step (ivc is the chunk idx)
                            xdma((ivc + k) * ch + jj + 1)

                chunk_pair(0)  # peeled
                assert nchunks >= 4
                with tc.For_i(2, nchunks, 2) as ivc:
                    chunk_pair(ivc)

                # epilogue: final h2(T-1) from last gates + cell state
                for s in range(2):
                    js = slice(s * HB, (s + 1) * HB)
                    nc.scalar.activation(thc[s][:], c2[:, js], AF.Tanh, scale=0.5)
                    nc.vector.scalar_tensor_tensor(
                        S[0:G1, js], th[s][:, 384:512], 1.0, thc[s][:],
                        op0=ALU.add, op1=ALU.mult,
                    )

            nc.sync.dma_start(feat_d[0:G1, :], S[0:G1, :])
            nc.sync.dma_start(fdbg[:], S[0:G1, :])

            # ---------------- stages 2 & 3, wavefronted ----------------
            with tc.tile_pool(name="psum", bufs=2, space="PSUM") as psum_pool:
                h2s = [state.tile([H2, BC], BF16, name=f"h2s_{p}") for p in range(2)]
                c2_ = state.tile([H2, BC], F32)
                h3s = [state.tile([H2, BC], BF16, name=f"h3s_{p}") for p in range(2)]
                c3_ = state.tile([H2, BC], F32)
                for t_ in h2s + h3s:
                    nc.vector.memset(t_[:], 0.0)
                nc.vector.memset(c2_[:], 0.0)
                nc.vector.memset(c3_[:], 0.0)
                x2blk = [
                    state.tile([2, 8 * BC], BF16, name=f"x2blk_{p}") for p in range(2)
                ]
                nc.vector.memset(x2blk[0][:], 1.0)  # row 0 re-DMA'd each block
                nc.vector.memset(x2blk[1][:], 1.0)

                def cell23(ps, c, h_out, pfx):
                    gt = work.tile([H2, BC], F32, name=f"gt{pfx}", tag="gt23")
                    nc.scalar.activation(
                        gt[:], ps[:, 3 * 512 : 3 * 512 + BC], AF.Tanh
                    )
                    s_all = work.tile([H2, 3 * BC], F32, name=f"s{pfx}", tag="s23")
                    nc.scalar.activation(
                        _r(s_all, "p (t b) -> p t b", b=BC),
                        _r(ps, "p (t b) -> p t b", b=512)[:, 0:3, 0:BC],
                        AF.Sigmoid,
                    )
                    u = work.tile([H2, BC], F32, name=f"u{pfx}", tag="u23")
                    nc.vector.tensor_mul(u[:], s_all[:, 0:BC], gt[:])
                    tm = work.tile([H2, BC], F32, name=f"tm{pfx}", tag="tm23")
                    nc.vector.tensor_mul(tm[:], s_all[:, BC : 2 * BC], c[:])
                    nc.vector.tensor_add(c[:], u[:], tm[:])
                    th_ = work.tile([H2, BC], F32, name=f"th{pfx}", tag="th23")
                    nc.scalar.activation(th_[:], c[:], AF.Tanh)
                    nc.vector.tensor_mul(h_out[:], s_all[:, 2 * BC :], th_[:])

                def step2(j, xpair):
                    ps = psum_pool.tile([128, 2048], F32, name="ps2", tag="ps")
                    for t in range(4):
                        mm(ps[:, t * 512 : t * 512 + BC], w2x[t], xpair, True, False)
                    for t in (3, 0, 1, 2):
                        mm(
                            ps[:, t * 512 : t * 512 + BC],
                            w2h[t],
                            h2s[j % 2],
                            False,
                            True,
                        )
                    cell23(ps, c2_, h2s[(j + 1) % 2], "2")

                def step3(j):
                    ps = psum_pool.tile([128, 2048], F32, name="ps3", tag="ps")
                    for t in range(4):
                        mm(ps[:, t * 512 : t * 512 + BC], b3[t], ones1, True, False)
                    for t in range(4):
                        mm(
                            ps[:, t * 512 : t * 512 + BC],
                            w3x[t],
                            h2s[(j + 1) % 2],
                            False,
                            False,
                        )
                    for t in (3, 0, 1, 2):
                        mm(
                            ps[:, t * 512 : t * 512 + BC],
                            w3h[t],
                            h3s[j % 2],
                            False,
                            True,
                        )
                    cell23(ps, c3_, h3s[(j + 1) % 2], "3")

                W2 = 16
                assert T2 % W2 == 0 and T2 >= 2 * W2
                nc.sync.dma_start(
                    x2blk[0][0:1, :], _r(feat_d[0:8, :], "r b -> (r b)")
                )
                nc.sync.dma_start(
                    x2blk[1][0:1, :], _r(feat_d[8:16, :], "r b -> (r b)")
                )

                def wave_block(ivw):
                    for half in range(2):
                        for jj in range(8):
                            j = 8 * half + jj
                            xpair = _r(
                                x2blk[half], "p (t b) -> p t b", b=BC
                            )[0:2, jj, :]
                            step2(j, xpair)
                            step3(j)
                        off = ivw + 16 + 8 * half
                        nc.sync.dma_start(
                            x2blk[half][0:1, :],
                            _r(feat_d[bass.ds(off, 8), :], "r b -> (r b)"),
                        )

                wave_block(0)  # peeled
                with tc.For_i(W2, T2, W2) as ivw:
                    wave_block(ivw)

                # ---------------- FC head ----------------
                h3f = h3s[T2 % 2]
                psf = psum_pool.tile([128, 2048], F32, name="psf", tag="ps")
                mm(psf[:, 0:BC], wf1, h3f, True, True)
                a1 = work.tile([H2, BC], BF16)
                nc.vector.tensor_scalar(
                    a1[:], psf[:, 0:BC], bf1, 0.0, op0=ALU.add, op1=ALU.max
                )
                mm(psf[:, 512 : 512 + BC], wf3, a1, True, True)
                a3 = work.tile([H2, BC], BF16)
                nc.vector.tensor_scalar(
                    a3[:], psf[:, 512 : 512 + BC], bf3, 0.0, op0=ALU.add, op1=ALU.max
                )
                mm(psf[0:2, 1024 : 1024 + BC], wf2, a3, True, True)
                yt = work.tile([2, BC], F32)
                nc.vector.tensor_scalar_add(yt[:], psf[0:2, 1024 : 1024 + BC], bf2)
                nc.sync.dma_start(y_t[:], yt[:])

    nc.compile()
    return nc


def _bf16(a):
    import ml_dtypes

    return np.asarray(a, np.float32).astype(ml_dtypes.bfloat16)


def pack_weights(i):
    f32 = np.float32
    wp = np.zeros((128, WBCOLS), f32)
    Wih, Whh = np.asarray(i["rnn_Wih"], f32), np.asarray(i["rnn_Whh"], f32)
    bb1 = np.asarray(i["rnn_bih"], f32) + np.asarray(i["rnn_bhh"], f32)
    # stage-1 stationaries: [SROWS, 96] per gate (i, f, o, g)
    for t, base in enumerate(GATE_BASES_1):
        k = 2.0 if t == 2 else 1.0  # g-gate (bank 2) doubled (call scale 0.5)
        o = W1_O + G1 * t
        blk = np.zeros((SROWS, G1), f32)
        for c in range(NCH):
            # h2 rows: Whh/2 block-diagonal
            blk[16 * c : 16 * c + 16, 16 * c : 16 * c + 16] = (
                Whh[base : base + H1, :].T * (0.5 * k)
            )
            blk[96 + c, 16 * c : 16 * c + 16] = Wih[base : base + H1, 0] * k
        blk[102, :] = np.tile(bb1[base : base + H1], NCH) * k
        wp[0:SROWS, o : o + G1] = blk
    # stage 2: x+bias K=2 stationaries; Wih0 halved (feat = 2h)
    bb2 = np.asarray(i["rnn2_bih0"], f32) + np.asarray(i["rnn2_bhh0"], f32)
    for t, base in enumerate(GATE_BASES_2):
        wp[0, W2X_O + H2 * t : W2X_O + H2 * (t + 1)] = (
            np.asarray(i["rnn2_Wih0"], f32)[base : base + H2, 0] * 0.5
        )
        wp[1, W2X_O + H2 * t : W2X_O + H2 * (t + 1)] = bb2[base : base + H2]
        wp[0:H2, W2H_O + H2 * t : W2H_O + H2 * (t + 1)] = np.asarray(
            i["rnn2_Whh0"], f32
        )[base : base + H2, :].T
    bb3 = np.asarray(i["rnn2_bih1"], f32) + np.asarray(i["rnn2_bhh1"], f32)
    for t, base in enumerate(GATE_BASES_2):
        wp[0:H2, W3X_O + H2 * t : W3X_O + H2 * (t + 1)] = np.asarray(
            i["rnn2_Wih1"], f32
        )[base : base + H2, :].T
        wp[0:H2, W3H_O + H2 * t : W3H_O + H2 * (t + 1)] = np.asarray(
            i["rnn2_Whh1"], f32
        )[base : base + H2, :].T
        wp[0, B3_O + H2 * t : B3_O + H2 * (t + 1)] = bb3[base : base + H2]
    wp[0:H2, WF1_O : WF1_O + H2] = np.asarray(i["fc1_W"], f32).T
    wp[0:H2, WF3_O : WF3_O + H2] = np.asarray(i["fc3_W"], f32).T
    wp[0:H2, WF2_O : WF2_O + 2] = np.asarray(i["fc2_W"], f32).T
    wp[0, ONES_O : ONES_O + BC] = 1.0

    wpf = np.zeros((128, 3), f32)
    wpf[0:H2, 0] = np.asarray(i["fc1_b"], f32)
    wpf[0:H2, 1] = np.asarray(i["fc3_b"], f32)
    wpf[0:2, 2] = np.asarray(i["fc2_b"], f32)
    return _bf16(wp), wpf


def make_in_maps(inputs, T1=T1_FULL, ch=CH):
    wp, wpf = pack_weights(inputs)
    x = np.asarray(inputs["x"], np.float32)
    tpad = 2 * ch
    maps = []
    for k in range(NCORES):
        xk = np.zeros((NCH, T1 + tpad, BC), np.float32)
        xk[:, :T1, :] = np.ascontiguousarray(
            x[k * BC : (k + 1) * BC, :T1, :].transpose(2, 1, 0)
        )
        maps.append({"xT": _bf16(xk), "wpack": wp, "wpf": wpf})
    return maps


def kernel(**inputs):
    from concourse.bass_utils import run_bass_kernel_spmd

    nc = build_program()
    in_maps = make_in_maps(inputs)
    res = run_bass_kernel_spmd(nc, in_maps, list(range(NCORES)))
    outs = [np.asarray(res.results[k]["yT"]) for k in range(NCORES)]
    return np.concatenate([o.T for o in outs], axis=0).astype(np.float32)


# revision 23
# speedup vs baseline: 1.0112x; 1.0112x over previous
"""Trainium2 Bass kernel for nn_DeRNN_4054449127979.

Network (per reference):
  stage1: 6 shared-weight single-channel LSTMs (hidden 16) over T=1024,
          folded as one LSTM on [B*6, T, 1]; keep last hidden -> feat [B, 96]
  stage2: LSTM(1 -> 128) over the 96 features as a sequence (return_seq)
  stage3: LSTM(128 -> 128) over those 96 steps; keep last hidden
  head:   relu(fc1) -> relu(fc3) -> fc2  -> [B, 2]

Sharding: pure data parallel over batch across 8 cores (B=2048 -> 256/core).

Stage-1 design:
  - all matmuls bf16 (HW runs fp32/f32r matmuls at 4 cyc/row; bf16 at 1)
  - x rows + ones row live inside the state tile -> ONE combined matmul
    per gate (K=103) instead of separate x/h matmuls; the per-step x copy
    runs in the tanh(c)/h window, off the critical chain
  - sigmoid-via-tanh: all gates + tanh(c) use ONE activation function
    (Tanh, scale=0.5): i,f,o weights plain (tanh(z/2) -> sigma=(t+1)/2),
    g-gate weights doubled, cell state stored as c2=2c, hidden stored as
    h2=2h with Whh halved.  One act table, 2 calls/step/stream.
  - cell math as fused scalar_tensor_tensor / tensor_scalar ops
  - batch split in two j-streams (cols 0:128 / 128:256) whose per-step
    chains interleave on the engines.
Stage 2/3: baseline wavefront structure, bf16 matmuls, bias folded into
the K=2 x-matmul (stage2).
"""

import sys

import numpy as np

sys.path.insert(0, "/opt/trn_rl_repo")

import concourse.bass as bass  # noqa: E402
import concourse.tile as tile  # noqa: E402
from concourse import bacc, mybir  # noqa: E402

F32 = mybir.dt.float32
BF16 = mybir.dt.bfloat16
AF = mybir.ActivationFunctionType
ALU = mybir.AluOpType

B = 2048
NCORES = 8
BC = B // NCORES  # 256
HB = BC // 2  # 128 per j-stream
NCH = 6
H1 = 16
G1 = NCH * H1  # 96
H2 = 128
T1_FULL = 1024
T2_FULL = 96
CH = 32  # stage-1 steps per staged x chunk
SROWS = 103  # state tile rows: 0:96 h2 | 96:102 x | 102 ones

# stage-1 gate bank order on chip: f, i, g, o ; torch rows are i, f, g, o
GATE_BASES_1 = (H1, 0, 2 * H1, 3 * H1)
# stage-2/3 bank order: i, f, o, g
GATE_BASES_2 = (0, H2, 3 * H2, 2 * H2)

# ---- packed bf16 weight column map ----
_off = 0


def _take(n):
    global _off
    o = _off
    _off += n
    return o


W1_O = _take(4 * G1)  # 4 gates x [SROWS, 96]
W2X_O = _take(4 * H2)  # rows 0:2 = [Wih0/2 ; b2]
W2H_O = _take(4 * H2)  # [128, 128] per gate
W3X_O = _take(4 * H2)  # [128, 128] per gate (Wih1^T)
W3H_O = _take(4 * H2)  # [128, 128] per gate
B3_O = _take(4 * H2)  # row 0 = b3 per gate
WF1_O = _take(H2)
WF3_O = _take(H2)
WF2_O = _take(2)
ONES_O = _take(BC)  # row 0 = ones
WBCOLS = _off


def _r(ap, pattern, **kw):
    return ap.rearrange(pattern, **kw)


def build_program(T1=T1_FULL, T2=T2_FULL, ch=CH):
    nc = bacc.Bacc("TRN2", target_bir_lowering=False)

    tpad = 2 * ch
    x_t = nc.declare_dram_parameter("xT", [NCH, T1 + tpad, BC], BF16, isOutput=False)
    wp_d = nc.declare_dram_parameter("wpack", [128, WBCOLS], BF16, isOutput=False)
    wf_d = nc.declare_dram_parameter("wpf", [128, 3], F32, isOutput=False)
    y_t = nc.declare_dram_parameter("yT", [2, BC], F32, isOutput=True)
    fdbg = nc.declare_dram_parameter("fdbg", [G1, BC], BF16, isOutput=True)

    feat_d = nc.dram_tensor("featstage", [G1 + 32, BC], BF16)

    def mm(out, lhsT, rhs, start, stop):
        nc.tensor.matmul(out, lhsT, rhs, start=start, stop=stop)

    with tile.TileContext(nc) as tc:
        with (
            tc.tile_pool(name="wpool", bufs=1) as wpool,
            tc.tile_pool(name="state", bufs=1) as state,
            tc.tile_pool(name="work", bufs=3) as work,
        ):
            wp = wpool.tile([128, WBCOLS], BF16)
            nc.sync.dma_start(wp[:], wp_d[:])
            wpf = wpool.tile([128, 3], F32)
            nc.sync.dma_start(wpf[:], wf_d[:])

            w1 = [wp[0:SROWS, W1_O + G1 * t : W1_O + G1 * (t + 1)] for t in range(4)]
            w2x = [wp[0:2, W2X_O + H2 * t : W2X_O + H2 * (t + 1)] for t in range(4)]
            w2h = [wp[0:H2, W2H_O + H2 * t : W2H_O + H2 * (t + 1)] for t in range(4)]
            w3x = [wp[0:H2, W3X_O + H2 * t : W3X_O + H2 * (t + 1)] for t in range(4)]
            w3h = [wp[0:H2, W3H_O + H2 * t : W3H_O + H2 * (t + 1)] for t in range(4)]
            b3 = [wp[0:1, B3_O + H2 * t : B3_O + H2 * (t + 1)] for t in range(4)]
            wf1 = wp[0:H2, WF1_O : WF1_O + H2]
            wf3 = wp[0:H2, WF3_O : WF3_O + H2]
            wf2 = wp[0:H2, WF2_O : WF2_O + 2]
            ones1 = wp[0:1, ONES_O : ONES_O + BC]
            bf1 = wpf[0:H2, 0:1]
            bf3 = wpf[0:H2, 1:2]
            bf2 = wpf[0:2, 2:3]

            # ---------------- stage 1 ----------------
            S = state.tile([SROWS, BC], BF16)  # h2 | x | ones
            nc.vector.memset(S[:], 0.0)
            # ones row: memset rows 96:103 (quadrant-aligned start); the
            # x rows 96:102 are overwritten by the first xcopy anyway
            nc.vector.memset(S[96:SROWS, :], 1.0)
            c2 = state.tile([G1, BC], F32)
            nc.vector.memset(c2[:], 0.0)
            th = [state.tile([G1, 512], BF16, name=f"th_{s}") for s in range(2)]
            thc = [state.tile([G1, HB], BF16, name=f"thc_{s}") for s in range(2)]
            for s in range(2):
                nc.vector.memset(th[s][:], 0.0)
                nc.vector.memset(thc[s][:], 0.0)
            tf2 = [work.tile([G1, HB], BF16, name=f"tf2_{s}", tag="tf2") for s in range(2)]
            p1t = [work.tile([G1, HB], BF16, name=f"p1_{s}", tag="p1") for s in range(2)]
            p2t = [work.tile([G1, HB], F32, name=f"p2_{s}", tag="p2") for s in range(2)]

            with tc.tile_pool(name="ps1pool", bufs=1, space="PSUM") as ps1pool:
                ps1 = [ps1pool.tile([128, 512], F32, name=f"ps1_{s}") for s in range(2)]
                xv = _r(x_t[:], "c t b -> c (t b)")

                def xdma(tidx):
                    # DMA x rows for step tidx straight into the state tile;
                    # completes in the thc/h2 window, off the matmul chain.
                    nc.sync.dma_start(
                        S[96:102, :], xv[:, bass.ds(tidx * BC, BC)]
                    )

                def substep(s):
                    # bank order: 0=f, 1=i, 2=g, 3=o
                    js = slice(s * HB, (s + 1) * HB)
                    # tanh(c(t-1)) for this stream (c2 = 2c, scale 0.5)
                    nc.scalar.activation(thc[s][:], c2[:, js], AF.Tanh, scale=0.5)
                    # h2(t-1) = (t_o + 1) * thc -> state rows (bf16)
                    nc.vector.scalar_tensor_tensor(
                        S[0:G1, js], th[s][:, 384:512], 1.0, thc[s][:],
                        op0=ALU.add, op1=ALU.mult,
                    )
                    # gates(t): 4 combined matmuls K=SROWS, N=HB; acts split
                    # [f,i] / [g,o] so the first tanh overlaps the g/o mms
                    for t4 in range(2):
                        mm(
                            ps1[s][0:G1, t4 * 128 : t4 * 128 + 128],
                            w1[t4],
                            S[0:SROWS, js],
                            True,
                            True,
                        )
                    nc.scalar.activation(
                        _r(th[s][:, 0:256], "p (t b) -> p t b", b=128),
                        _r(ps1[s][0:G1, 0:256], "p (t b) -> p t b", b=128),
                        AF.Tanh,
                        scale=0.5,
                    )
                    for t4 in range(2, 4):
                        mm(
                            ps1[s][0:G1, t4 * 128 : t4 * 128 + 128],
                            w1[t4],
                            S[0:SROWS, js],
                            True,
                            True,
                        )
                    nc.scalar.activation(
                        _r(th[s][:, 256:512], "p (t b) -> p t b", b=128),
                        _r(ps1[s][0:G1, 256:512], "p (t b) -> p t b", b=128),
                        AF.Tanh,
                        scale=0.5,
                    )
                    # cell: sigma_f = tf*0.5+0.5 (Pool) ; P1 = (ti+1)*tg ;
                    # P2 = sigma_f * c2 ; c2' = P1 + P2
                    nc.gpsimd.tensor_scalar(
                        tf2[s][:], th[s][:, 0:128], 0.5, 0.5,
                        op0=ALU.mult, op1=ALU.add,
                    )
                    nc.vector.scalar_tensor_tensor(
                        p1t[s][:], th[s][:, 128:256], 1.0, th[s][:, 256:384],
                        op0=ALU.add, op1=ALU.mult,
                    )
                    nc.vector.tensor_tensor(
                        p2t[s][:], tf2[s][:], c2[:, js], op=ALU.mult
                    )
                    nc.vector.tensor_tensor(
                        c2[:, js], p1t[s][:], p2t[s][:], op=ALU.add
                    )

                nchunks = T1 // ch
                xdma(0)  # x rows for step 0

                def chunk_pair(ivc):
                    for k in range(2):
                        for jj in range(ch):
                            substep(0)
                            substep(1)
                            # x rows for the next step (ivc is the chunk idx)
                            xdma((ivc + k) * ch + jj + 1)

                chunk_pair(0)  # peeled
                assert nchunks >= 4
                with tc.For_i(2, nchunks, 2) as ivc:
                    chunk_pair(ivc)

                # epilogue: final h2(T-1) from last gates + cell state
                for s in range(2):
                    js = slice(s * HB, (s + 1) * HB)
                    nc.scalar.activation(thc[s][:], c2[:, js], AF.Tanh, scale=0.5)
                    nc.vector.scalar_tensor_tensor(
                        S[0:G1, js], th[s][:, 384:512], 1.0, thc[s][:],
                        op0=ALU.add, op1=ALU.mult,
                    )

            nc.sync.dma_start(feat_d[0:G1, :], S[0:G1, :])
            nc.sync.dma_start(fdbg[:], S[0:G1, :])

            # ---------------- stages 2 & 3, wavefronted ----------------
            with tc.tile_pool(name="psum", bufs=2, space="PSUM") as psum_pool:
                h2s = [state.tile([H2, BC], BF16, name=f"h2s_{p}") for p in range(2)]
                c2_ = state.tile([H2, BC], F32)
                h3s = [state.tile([H2, BC], BF16, name=f"h3s_{p}") for p in range(2)]
                c3_ = state.tile([H2, BC], F32)
                for t_ in h2s + h3s:
                    nc.vector.memset(t_[:], 0.0)
                nc.vector.memset(c2_[:], 0.0)
                nc.vector.memset(c3_[:], 0.0)
                x2blk = [
                    state.tile([2, 8 * BC], BF16, name=f"x2blk_{p}") for p in range(2)
                ]
                nc.vector.memset(x2blk[0][:], 1.0)  # row 0 re-DMA'd each block
                nc.vector.memset(x2blk[1][:], 1.0)

                def cell23(ps, c, h_out, pfx):
                    gt = work.tile([H2, BC], F32, name=f"gt{pfx}", tag="gt23")
                    nc.scalar.activation(
                        gt[:], ps[:, 3 * 512 : 3 * 512 + BC], AF.Tanh
                    )
                    s_all = work.tile([H2, 3 * BC], F32, name=f"s{pfx}", tag="s23")
                    nc.scalar.activation(
                        _r(s_all, "p (t b) -> p t b", b=BC),
                        _r(ps, "p (t b) -> p t b", b=512)[:, 0:3, 0:BC],
                        AF.Sigmoid,
                    )
                    u = work.tile([H2, BC], F32, name=f"u{pfx}", tag="u23")
                    nc.vector.tensor_mul(u[:], s_all[:, 0:BC], gt[:])
                    tm = work.tile([H2, BC], F32, name=f"tm{pfx}", tag="tm23")
                    nc.vector.tensor_mul(tm[:], s_all[:, BC : 2 * BC], c[:])
                    nc.vector.tensor_add(c[:], u[:], tm[:])
                    th_ = work.tile([H2, BC], F32, name=f"th{pfx}", tag="th23")
                    nc.scalar.activation(th_[:], c[:], AF.Tanh)
                    nc.vector.tensor_mul(h_out[:], s_all[:, 2 * BC :], th_[:])

                def step2(j, xpair):
                    ps = psum_pool.tile([128, 2048], F32, name="ps2", tag="ps")
                    for t in range(4):
                        mm(ps[:, t * 512 : t * 512 + BC], w2x[t], xpair, True, False)
                    for t in (3, 0, 1, 2):
                        mm(
                            ps[:, t * 512 : t * 512 + BC],
                            w2h[t],
                            h2s[j % 2],
                            False,
                            True,
                        )
                    cell23(ps, c2_, h2s[(j + 1) % 2], "2")

                def step3(j):
                    ps = psum_pool.tile([128, 2048], F32, name="ps3", tag="ps")
                    for t in range(4):
                        mm(ps[:, t * 512 : t * 512 + BC], b3[t], ones1, True, False)
                    for t in range(4):
                        mm(
                            ps[:, t * 512 : t * 512 + BC],
                            w3x[t],
                            h2s[(j + 1) % 2],
                            False,
                            False,
                        )
                    for t in (3, 0, 1, 2):
                        mm(
                            ps[:, t * 512 : t * 512 + BC],
                            w3h[t],
                            h3s[j % 2],
                            False,
                            True,
                        )
                    cell23(ps, c3_, h3s[(j + 1) % 2], "3")

                W2 = 16
                assert T2 % W2 == 0 and T2 >= 2 * W2
                nc.sync.dma_start(
                    x2blk[0][0:1, :], _r(feat_d[0:8, :], "r b -> (r b)")
                )
                nc.sync.dma_start(
                    x2blk[1][0:1, :], _r(feat_d[8:16, :], "r b -> (r b)")
                )

                def wave_block(ivw):
                    for half in range(2):
                        for jj in range(8):
                            j = 8 * half + jj
                            xpair = _r(
                                x2blk[half], "p (t b) -> p t b", b=BC
                            )[0:2, jj, :]
                            step2(j, xpair)
                            step3(j)
                        off = ivw + 16 + 8 * half
                        nc.sync.dma_start(
                            x2blk[half][0:1, :],
                            _r(feat_d[bass.ds(off, 8), :], "r b -> (r b)"),
                        )

                wave_block(0)  # peeled
                with tc.For_i(W2, T2, W2) as ivw:
                    wave_block(ivw)

                # ---------------- FC head ----------------
                h3f = h3s[T2 % 2]
                psf = psum_pool.tile([128, 2048], F32, name="psf", tag="ps")
                mm(psf[:, 0:BC], wf1, h3f, True, True)
                a1 = work.tile([H2, BC], BF16)
                nc.vector.tensor_scalar(
                    a1[:], psf[:, 0:BC], bf1, 0.0, op0=ALU.add, op1=ALU.max
                )
                mm(psf[:, 512 : 512 + BC], wf3, a1, True, True)
                a3 = work.tile([H2, BC], BF16)
                nc.vector.tensor_scalar(
                    a3[:], psf[:, 512 : 512 + BC], bf3, 0.0, op0=ALU.add, op1=ALU.max
                )
                mm(psf[0:2, 1024 : 1024 + BC], wf2, a3, True, True)
                yt = work.tile([2, BC], F32)
                nc.vector.tensor_scalar_add(yt[:], psf[0:2, 1024 : 1024 + BC], bf2)
                nc.sync.dma_start(y_t[:], yt[:])

    nc.compile()
    return nc


def _bf16(a):
    import ml_dtypes

    return np.asarray(a, np.float32).astype(ml_dtypes.bfloat16)


def pack_weights(i):
    f32 = np.float32
    wp = np.zeros((128, WBCOLS), f32)
    Wih, Whh = np.asarray(i["rnn_Wih"], f32), np.asarray(i["rnn_Whh"], f32)
    bb1 = np.asarray(i["rnn_bih"], f32) + np.asarray(i["rnn_bhh"], f32)
    # stage-1 stationaries: [SROWS, 96] per gate (i, f, o, g)
    for t, base in enumerate(GATE_BASES_1):
        k = 2.0 if t == 2 else 1.0  # g-gate (bank 2) doubled (call scale 0.5)
        o = W1_O + G1 * t
        blk = np.zeros((SROWS, G1), f32)
        for c in range(NCH):
            # h2 rows: Whh/2 block-diagonal
            blk[16 * c : 16 * c + 16, 16 * c : 16 * c + 16] = (
                Whh[base : base + H1, :].T * (0.5 * k)
            )
            blk[96 + c, 16 * c : 16 * c + 16] = Wih[base : base + H1, 0] * k
        blk[102, :] = np.tile(bb1[base : base + H1], NCH) * k
        wp[0:SROWS, o : o + G1] = blk
    # stage 2: x+bias K=2 stationaries; Wih0 halved (feat = 2h)
    bb2 = np.asarray(i["rnn2_bih0"], f32) + np.asarray(i["rnn2_bhh0"], f32)
    for t, base in enumerate(GATE_BASES_2):
        wp[0, W2X_O + H2 * t : W2X_O + H2 * (t + 1)] = (
            np.asarray(i["rnn2_Wih0"], f32)[base : base + H2, 0] * 0.5
        )
        wp[1, W2X_O + H2 * t : W2X_O + H2 * (t + 1)] = bb2[base : base + H2]
        wp[0:H2, W2H_O + H2 * t : W2H_O + H2 * (t + 1)] = np.asarray(
            i["rnn2_Whh0"], f32
        )[base : base + H2, :].T
    bb3 = np.asarray(i["rnn2_bih1"], f32) + np.asarray(i["rnn2_bhh1"], f32)
    for t, base in enumerate(GATE_BASES_2):
        wp[0:H2, W3X_O + H2 * t : W3X_O + H2 * (t + 1)] = np.asarray(
            i["rnn2_Wih1"], f32
        )[base : base + H2, :].T
        wp[0:H2, W3H_O + H2 * t : W3H_O + H2 * (t + 1)] = np.asarray(
            i["rnn2_Whh1"], f32
        )[base : base + H2, :].T
        wp[0, B3_O + H2 * t : B3_O + H2 * (t + 1)] = bb3[base : base + H2]
    wp[0:H2, WF1_O : WF1_O + H2] = np.asarray(i["fc1_W"], f32).T
    wp[0:H2, WF3_O : WF3_O + H2] = np.asarray(i["fc3_W"], f32).T
    wp[0:H2, WF2_O : WF2_O + 2] = np.asarray(i["fc2_W"], f32).T
    wp[0, ONES_O : ONES_O + BC] = 1.0

    wpf = np.zeros((128, 3), f32)
    wpf[0:H2, 0] = np.asarray(i["fc1_b"], f32)
    wpf[0:H2, 1] = np.asarray(i["fc3_b"], f32)
    wpf[0:2, 2] = np.asarray(i["fc2_b"], f32)
    return _bf16(wp), wpf


def make_in_maps(inputs, T1=T1_FULL, ch=CH):
    wp, wpf = pack_weights(inputs)
    x = np.asarray(inputs["x"], np.float32)
    tpad = 2 * ch
    maps = []
    for k in range(NCORES):
        xk = np.zeros((NCH, T1 + tpad, BC), np.float32)
        xk[:, :T1, :] = np.ascontiguousarray(
            x[k * BC : (k + 1) * BC, :T1, :].transpose(2, 1, 0)
        )
        maps.append({"xT": _bf16(xk), "wpack": wp, "wpf": wpf})
    return maps


def kernel(**inputs):
    from concourse.bass_utils import run_bass_kernel_spmd

    nc = build_program()
    in_maps = make_in_maps(inputs)
    res = run_bass_kernel_spmd(nc, in_maps, list(range(NCORES)))
    outs = [np.asarray(res.results[k]["yT"]) for k in range(NCORES)]
    return np.concatenate([o.T for o in outs], axis=0).astype(np.float32)
